# revision 10
# baseline (speedup 1.0000x reference)
"""Trainium2 Bass kernel for CausalSelfAttention (GQA + RoPE + QK-RMSNorm).

Sharding: 8 cores = DP(2 batches) x TP(4 head-groups).
Core c handles batch b=c//4, head group g=c%4 (q heads 4g..4g+3, kv head g).
Device: QKV proj (bf16 in, fp32 accum), RoPE+RMSNorm, causal attention
(max-free softmax — QK-norm bounds |score| <= sqrt(D)), PV in bf16 with a
ones-column appended to V so row-sums come out of the same matmul, per-512-
column AllGather of the transposed attention output across each 4-core group
(overlapped with the output projection), out-proj for this group's 512 output
channels. Host concatenates the 8 per-core [T, 512] results into [B, T, C].
"""

import sys
import numpy as np

for _p in ("/opt/trn_rl_repo", "/root/.axon_site/_ro/trn_rl_repo"):
    if _p not in sys.path:
        sys.path.append(_p)

import ml_dtypes

import concourse.bass as bass
import concourse.mybir as mybir
import concourse.tile as tile
from concourse import bacc
from concourse.bass_utils import run_bass_kernel_spmd
from concourse.masks import make_identity

F32 = mybir.dt.float32
F32R = mybir.dt.float32r
BF16 = mybir.dt.bfloat16
AF = mybir.ActivationFunctionType
ALU = mybir.AluOpType
BF16NP = ml_dtypes.bfloat16

B, T, C = 2, 2048, 2048
H, KVH, D = 16, 4, 128
HLOC = H // 4          # q heads per core (TP=4)
DH = HLOC * D          # 512 output channels per core
EPS = 1.1920929e-07
N_CORES = 8

TT = 512               # t-tile (moving free dim)
# dev knobs (not used by the grading path)
CFG = {"skip_ag": False, "phases": 3, "trace_sim": False}
NKC = C // 128         # 16 contraction chunks for the projections
SM_SCALE = float(1.0 / np.sqrt(float(D)))


def build_nc(t_seq=T, n_reps=1):
    """Build the SPMD program for one core (all cores run the same program).

    n_reps > 1 replicates the whole body for on-device timing (slope method).
    """
    nc = bacc.Bacc("TRN2", target_bir_lowering=False, debug=False,
                   num_devices=N_CORES)

    xT = nc.dram_tensor("xT", [C, t_seq], BF16, kind="ExternalInput").ap()
    wqT = nc.dram_tensor("wqT", [C, DH], BF16, kind="ExternalInput").ap()
    wkT = nc.dram_tensor("wkT", [C, D], BF16, kind="ExternalInput").ap()
    wvT = nc.dram_tensor("wvT", [C, D], BF16, kind="ExternalInput").ap()
    woT = nc.dram_tensor("woT", [C, DH], BF16, kind="ExternalInput").ap()
    cos2 = nc.dram_tensor("cos2", [D, t_seq], F32, kind="ExternalInput").ap()
    sin2s = nc.dram_tensor("sin2s", [D, t_seq], F32, kind="ExternalInput").ap()
    out = nc.dram_tensor("out", [t_seq, DH], F32, kind="ExternalOutput").ap()

    groups = [[0, 1, 2, 3], [4, 5, 6, 7]]

    with tile.TileContext(nc, trace_sim=CFG["trace_sim"]) as tc:
        for _ in range(n_reps):
            build_body(tc, nc, xT, wqT, wkT, wvT, woT, cos2, sin2s, out,
                       groups, t_seq)
    nc.compile()
    return nc


def build_body(tc, nc, xT, wqT, wkT, wvT, woT, cos2, sin2s, out,
               groups, t_seq):
    from contextlib import ExitStack

    tt = TT
    nt = t_seq // tt       # t tiles
    nkb = t_seq // 128     # key blocks

    ctx = ExitStack()
    with ctx:
        # ---------- persistent pools ----------
        const_pool = ctx.enter_context(tc.tile_pool(name="const", bufs=1))
        qk_pool = ctx.enter_context(tc.tile_pool(name="qk", bufs=1))
        yv_pool = ctx.enter_context(tc.tile_pool(name="yv", bufs=1))
        dram = ctx.enter_context(tc.tile_pool(name="dram", bufs=1, space="DRAM"))

        ident = const_pool.tile([128, 128], F32, name="ident")
        make_identity(nc, ident[:])
        ident_bf = const_pool.tile([128, 128], BF16, name="ident_bf")
        nc.scalar.activation(ident_bf[:], ident[:], AF.Copy)
        ones_f = const_pool.tile([128, 1], F32, name="ones_f")
        nc.gpsimd.memset(ones_f[:], 1.0)
        ones_col = const_pool.tile([128, 1], F32R, name="ones_col")
        nc.scalar.activation(ones_col[:], ones_f[:], AF.Copy)
        ones_row = const_pool.tile([1, 128], F32R, name="ones_row")
        nc.scalar.activation(ones_row[:], ones_f[0:1, :].to_broadcast([1, 128]),
                             AF.Copy)
        eps_t = const_pool.tile([1, 1], F32, name="eps_t")
        nc.gpsimd.memset(eps_t[:], EPS)
        smsc_f = const_pool.tile([1, 1], F32, name="smsc_f")
        nc.gpsimd.memset(smsc_f[:], SM_SCALE)

        # qT/kT normalized+roped, [D, t_seq] per head (bf16)
        qTn = [qk_pool.tile([128, t_seq], BF16, name=f"qTn{h}") for h in range(HLOC)]
        kTn = qk_pool.tile([128, t_seq], BF16, name="kTn")
        # per-key-block exp scales: [128, 1] = SM_SCALE / rms(k)[tk]
        rks = [yv_pool.tile([128, 1], F32, name=f"rks{j}") for j in range(nkb)]
        # v_aug: per key block, [128 tk, 129] bf16 (col 128 = 1.0)
        v_aug = [yv_pool.tile([128, 129], BF16, name=f"vaug{j}") for j in range(nkb)]
        # attention output transposed: HLOC head-chunks of [128 c, t_seq]
        yT = [yv_pool.tile([128, t_seq], BF16, name=f"yT{h}") for h in range(HLOC)]

        # ================= phase 1: QKV projections =================
        with (
            tc.tile_pool(name="p1x", bufs=1) as p1x,
            tc.tile_pool(name="p1w", bufs=1) as p1w,
            tc.tile_pool(name="p1t", bufs=2) as p1t,
            tc.tile_pool(name="p1ps", bufs=4, space="PSUM") as p1ps,
            tc.tile_pool(name="p1sw", bufs=2, space="PSUM") as p1sw,
            tc.tile_pool(name="p1ss", bufs=1, space="PSUM") as p1ss,
        ):
            # weights (transposed, c-major) stay resident for phase 1.
            # Interleave x-chunk-0 and weight DMAs per c so the first
            # projection matmul can start after ~0.25 MB instead of ~5 MB.
            wq_sb = [p1w.tile([128, DH], BF16, name=f"wq{c}") for c in range(NKC)]
            wk_sb = [p1w.tile([128, D], BF16, name=f"wk{c}") for c in range(NKC)]
            wv_sb = [p1w.tile([128, D], BF16, name=f"wv{c}") for c in range(NKC)]
            cos_sb = p1w.tile([128, t_seq], F32, name="cos_sb")
            sin_sb = p1w.tile([128, t_seq], F32, name="sin_sb")
            nc.gpsimd.dma_start(cos_sb[:], cos2[:])
            nc.gpsimd.dma_start(sin_sb[:], sin2s[:])
            xt0 = []
            for c in range(NKC):
                xc = p1x.tile([128, tt], BF16, name=f"xt{c}", tag="xt",
                              bufs=NKC + 8)
                nc.sync.dma_start(xc[:], xT[128 * c:128 * (c + 1), 0:tt])
                xt0.append(xc)
                nc.sync.dma_start(wq_sb[c][:], wqT[128 * c:128 * (c + 1), :])
            for c in range(NKC):
                nc.sync.dma_start(wk_sb[c][:], wkT[128 * c:128 * (c + 1), :])
                nc.sync.dma_start(wv_sb[c][:], wvT[128 * c:128 * (c + 1), :])
            vT = p1w.tile([128, t_seq], BF16, name="vT")

            for i in range(nt):
                ts = slice(i * tt, (i + 1) * tt)
                # x^T chunk [C, tt] as NKC tiles of [128, tt]
                if i == 0:
                    xt = xt0
                else:
                    xt = []
                    for c in range(NKC):
                        xc = p1x.tile([128, tt], BF16, name=f"xt{c}", tag="xt",
                                      bufs=NKC + 8)
                        nc.sync.dma_start(xc[:], xT[128 * c:128 * (c + 1), ts])
                        xt.append(xc)

                for h in range(HLOC):      # q heads: rope + rmsnorm
                    ps = p1ps.tile([128, tt], F32, name="qkv_ps")
                    for c in range(NKC):
                        nc.tensor.matmul(
                            ps[:], wq_sb[c][:, 128 * h:128 * (h + 1)],
                            xt[c][:], start=(c == 0), stop=(c == NKC - 1))
                    rope_norm(nc, p1t, p1sw, p1ss, ps,
                              cos_sb[:, ts], sin_sb[:, ts],
                              qTn[h][:, ts], ones_col, ones_row, eps_t)
                # k head: rope, then 1/rms as per-tk exp scale (not applied
                # to kTn itself — folded into the softmax exp)
                ps = p1ps.tile([128, tt], F32, name="qkv_ps")
                for c in range(NKC):
                    nc.tensor.matmul(ps[:], wk_sb[c][:], xt[c][:],
                                     start=(c == 0), stop=(c == NKC - 1))
                rope_only(nc, p1t, p1sw, p1ss, ps,
                          cos_sb[:, ts], sin_sb[:, ts], kTn[:, ts],
                          ones_col, eps_t, smsc_f,
                          [rks[j] for j in range(4 * i, min(4 * i + 4, nkb))])
                # v head (no rope/norm)
                ps = p1ps.tile([128, tt], F32, name="qkv_ps")
                for c in range(NKC):
                    nc.tensor.matmul(ps[:], wv_sb[c][:], xt[c][:],
                                     start=(c == 0), stop=(c == NKC - 1))
                nc.scalar.activation(vT[:, ts], ps[:], AF.Copy)

                # v_aug for this chunk: transpose to [tk, d], cast bf16
                for j in range(4 * i, min(4 * i + 4, nkb)):
                    tp = p1sw.tile([128, 128], BF16, name="v_tp", tag="sw_ps",
                                   bufs=2)
                    nc.tensor.matmul(tp[:], vT[:, 128 * j:128 * (j + 1)],
                                     ident_bf[:], is_transpose=True)
                    nc.gpsimd.memset(v_aug[j][:, 128:129], 1.0)
                    nc.scalar.activation(v_aug[j][:, 0:128], tp[:], AF.Copy)

        # ================= phase 2+3 shared SBUF =================
        with tc.tile_pool(name="p2m", bufs=1) as p2m:
            masks = []
            for r in range(4):
                m = p2m.tile([128, tt], BF16, name=f"mask{r}")
                nc.gpsimd.memset(m[:], 1.0)
                nc.gpsimd.affine_select(
                    out=m[:], in_=m[:], compare_op=ALU.is_ge, fill=0.0,
                    base=-128 * r, pattern=[[1, tt]], channel_multiplier=-1)
                masks.append(m)

            # wo tiles (DMA overlaps with attention)
            wo_sb = [p2m.tile([128, DH], BF16, name=f"wo{c}") for c in range(NKC)]
            for c in range(NKC):
                nc.gpsimd.dma_start(wo_sb[c][:], woT[128 * c:128 * (c + 1), :])

            # last tile's AG is split into two head-pair halves so the
            # final out-proj can start before the last heads finish
            ag_in = [dram.tile([DH, tt], BF16, name=f"ag_in{i}")
                     for i in range(nt - 1)]
            ag_out = [dram.tile([4 * DH, tt], BF16, name=f"ag_out{i}")
                      for i in range(nt - 1)]
            ag_in3 = [dram.tile([2 * 128, tt], BF16, name=f"ag_in3{p}")
                      for p in range(2)]
            ag_out3 = [dram.tile([4 * 2 * 128, tt], BF16, name=f"ag_out3{p}")
                       for p in range(2)]

            # ---------- phase 2: causal attention ----------
            with (
                tc.tile_pool(name="p2pt", bufs=6) as p2pt,
                tc.tile_pool(name="p2on", bufs=6) as p2on,
                tc.tile_pool(name="p2st", bufs=2, space="PSUM") as p2st,
                tc.tile_pool(name="p2o", bufs=1, space="PSUM") as p2o,
                tc.tile_pool(name="p2tp", bufs=1, space="PSUM") as p2tp,
                tc.tile_pool(name="p3y", bufs=1) as p3y,
                tc.tile_pool(name="p3t", bufs=6) as p3t,
            ):
                for i in range(nt if CFG["phases"] >= 2 else 0):
                    nj = min(4 * i + 4, nkb)
                    for h in range(HLOC):
                        # token-blocks t=0,1 and t=2,3 pair up in one PSUM
                        # bank each, as a single accumulation group per bank
                        o01 = p2o.tile([128, 258], F32, name="o01", tag="o01",
                                       bufs=2)
                        o23 = p2o.tile([128, 258], F32, name="o23", tag="o23",
                                       bufs=2)
                        o_of = {0: (o01, 0), 1: (o01, 129),
                                2: (o23, 0), 3: (o23, 129)}
                        for j in range(nj):
                            r = j - 4 * i
                            # diagonal blocks r=1,2,3: the first 128*r score
                            # columns are fully masked — skip them
                            off = 128 * r if r > 0 else 0
                            npr = tt - off
                            st = p2st.tile([128, tt], F32, name="st_ps")
                            nc.tensor.matmul(
                                st[:, 0:npr], kTn[:, 128 * j:128 * (j + 1)],
                                qTn[h][:, i * tt + off:(i + 1) * tt])
                            pt = p2pt.tile([128, tt], BF16, name="pt")
                            nc.scalar.activation(pt[:, 0:npr], st[:, 0:npr],
                                                 AF.Exp, scale=rks[j][:])
                            if r >= 0:
                                nc.vector.tensor_mul(pt[:, 0:npr], pt[:, 0:npr],
                                                     masks[r][:, off:tt])
                            for t in range(4):
                                if j <= 4 * i + t:
                                    ot_ps, oc = o_of[t]
                                    nc.tensor.matmul(
                                        ot_ps[:, oc:oc + 129],
                                        pt[:, 128 * t - off:128 * (t + 1) - off],
                                        v_aug[j][:],
                                        start=(j == 0 and t % 2 == 0),
                                        stop=(j == 4 * i + t and t % 2 == 1))
                        for t in range(4):
                            ot_ps, oc = o_of[t]
                            rec = p2on.tile([128, 1], F32, name="rec")
                            nc.vector.reciprocal(
                                rec[:], ot_ps[:, oc + 128:oc + 129])
                            o_n = p2on.tile([128, 128], BF16, name="o_n")
                            nc.vector.tensor_scalar_mul(
                                o_n[:], ot_ps[:, oc:oc + 128], rec[:])
                            tp = p2tp.tile([128, 128], BF16, name="o_tp")
                            nc.tensor.matmul(tp[:], o_n[:], ident_bf[:],
                                             is_transpose=True)
                            nc.vector.tensor_copy(
                                yT[h][:, i * tt + 128 * t:i * tt + 128 * (t + 1)],
                                tp[:])
                        if i == nt - 1:
                            # eager per-head staging; AG per head-pair
                            pair, ha = h // 2, h % 2
                            nc.sync.dma_start(
                                ag_in3[pair][128 * ha:128 * (ha + 1), :],
                                yT[h][:, i * tt:(i + 1) * tt])
                            if ha == 1 and not CFG["skip_ag"]:
                                nc.gpsimd.collective_compute(
                                    "AllGather", ALU.bypass,
                                    replica_groups=groups,
                                    ins=[ag_in3[pair][:]],
                                    outs=[ag_out3[pair][:]])
                    if i < nt - 1:
                        # yT chunk i complete for all heads -> stage + AllGather
                        for h in range(HLOC):
                            nc.sync.dma_start(ag_in[i][128 * h:128 * (h + 1), :],
                                              yT[h][:, i * tt:(i + 1) * tt])
                        if not CFG["skip_ag"]:
                            nc.gpsimd.collective_compute(
                                "AllGather", ALU.bypass, replica_groups=groups,
                                ins=[ag_in[i][:]], outs=[ag_out[i][:]])

                # ---------- phase 3: output projection ----------
                for i in range(nt if CFG["phases"] >= 3 else 0):
                    if i < nt - 1:
                        order = list(range(NKC))
                        srcs = [ag_out[i][128 * c:128 * (c + 1), :]
                                for c in order]
                    else:
                        # wave A (c%4 in 0,1 — from ag_out3[0]) first, so
                        # the final accumulation can start before AG3b lands
                        order = ([c for c in range(NKC) if c % 4 < 2]
                                 + [c for c in range(NKC) if c % 4 >= 2])
                        srcs = []
                        for c in order:
                            pair, hl = (0, c % 4) if c % 4 < 2 else (1, c % 4 - 2)
                            base = 256 * (c // 4) + 128 * hl
                            srcs.append(ag_out3[pair][base:base + 128, :])
                    yf = {}
                    for c, src in zip(order, srcs):
                        yc = p3y.tile([128, tt], BF16, name=f"yf{c}", tag="yf",
                                      bufs=2 * NKC)
                        nc.sync.dma_start(yc[:], src)
                        yf[c] = yc
                    for t in range(4):
                        ps = p2st.tile([128, DH], F32, name="out_ps", tag="cp",
                                       bufs=1)
                        for k, c in enumerate(order):
                            nc.tensor.matmul(
                                ps[:], yf[c][:, 128 * t:128 * (t + 1)],
                                wo_sb[c][:],
                                start=(k == 0), stop=(k == NKC - 1))
                        ot = p3t.tile([128, DH], F32, name="ot")
                        nc.vector.tensor_copy(ot[:], ps[:])
                        nc.sync.dma_start(
                            out[i * tt + 128 * t:i * tt + 128 * (t + 1), :],
                            ot[:])


def rope_only(nc, tmp_pool, sw_pool, ss_pool, ps, cos, sin_s, out_ap,
              ones_col, eps_t, smsc, rks_blocks):
    """RoPE for k; writes roped (unnormalized) k to out_ap and the per-tk
    exp scales SM_SCALE/rms into rks_blocks ([128,1] each, via PE transpose
    of the [1, tt] reciprocal-rms row).

    RoPE is orthogonal per (t, d-pair), so rms(rope(x)) = rms(x): the rms
    path reads the pre-rope PSUM directly and runs concurrently with the
    rotation. The half-swap is two 64-partition DVE muls at partition
    offsets (no PE swap matmul, no staging copy).
    """
    ttl = ps.shape[-1]
    h2 = 64
    # rms path (from pre-rope ps)
    sq = tmp_pool.tile([128, ttl], F32R, name="sq", tag="sq")
    nc.scalar.activation(sq[:], ps[:], AF.Square)
    ss = ss_pool.tile([1, ttl], F32, name="ss_ps", tag="ss")
    nc.tensor.matmul(ss[:], ones_col[:], sq[:])
    sd = tmp_pool.tile([1, ttl], F32, name="sd", tag="sd")
    nc.scalar.activation(sd[:], ss[:], AF.Sqrt, scale=1.0 / 128.0,
                         bias=eps_t[:])
    rr = tmp_pool.tile([1, ttl], F32, name="rr", tag="rr")
    nc.vector.reciprocal(rr[:], sd[:])
    for b, rk in enumerate(rks_blocks):
        rkp = ss_pool.tile([128, 1], F32, name="rk_ps", tag="rb")
        nc.tensor.matmul(rkp[:], rr[0:1, 128 * b:128 * (b + 1)], smsc[:])
        nc.vector.tensor_copy(rk[:], rkp[:])
    # rotation path
    e1 = tmp_pool.tile([128, ttl], F32, name="e1", tag="e1")
    nc.vector.tensor_mul(e1[:], ps[:], cos)
    qr = tmp_pool.tile([128, ttl], F32, name="qr", tag="qr")
    nc.vector.tensor_mul(qr[0:h2, :], ps[h2:128, :], sin_s[0:h2, :])
    nc.vector.tensor_mul(qr[h2:128, :], ps[0:h2, :], sin_s[h2:128, :])
    nc.gpsimd.tensor_add(out_ap, e1[:], qr[:])


def rope_norm(nc, tmp_pool, sw_pool, ss_pool, ps, cos, sin_s, out_ap,
              ones_col, ones_row, eps_t):
    """RoPE + RMS-norm. ps: [128 d, tt] PSUM (pre-rope head), out_ap: SBUF.

    cos is [cos; cos] (rows duplicated), sin_s is [sin; -sin]. Rotation
    preserves the per-t norm, so the rms factor is computed from the
    pre-rope PSUM concurrently with the rotation, then applied once at
    the end: out = (ps*cos + swap(ps)*sin_s) * rsqrt(mean(ps^2)+eps).
    """
    ttl = ps.shape[-1]
    h2 = 64
    # rms path: sumsq over d via PE, rsqrt on ACT, bcast to 128 parts via PE
    sq = tmp_pool.tile([128, ttl], F32R, name="sq", tag="sq")
    nc.scalar.activation(sq[:], ps[:], AF.Square)
    ss = ss_pool.tile([1, ttl], F32, name="ss_ps", tag="ss")
    nc.tensor.matmul(ss[:], ones_col[:], sq[:])
    sd = tmp_pool.tile([1, ttl], F32R, name="sd", tag="sd")
    nc.scalar.activation(sd[:], ss[:], AF.Sqrt, scale=1.0 / 128.0,
                         bias=eps_t[:])
    rb = ss_pool.tile([128, ttl], F32, name="rb_ps", tag="rb")
    nc.tensor.matmul(rb[:], ones_row[:], sd[:])
    rec = tmp_pool.tile([128, ttl], F32, name="rec", tag="rec")
    nc.vector.reciprocal(rec[:], rb[:])
    # rotation path (reads PSUM directly; half-swap via partition offsets)
    e1 = tmp_pool.tile([128, ttl], F32, name="e1", tag="e1")
    nc.vector.tensor_mul(e1[:], ps[:], cos)
    qr = tmp_pool.tile([128, ttl], F32, name="qr", tag="qr")
    nc.vector.tensor_mul(qr[0:h2, :], ps[h2:128, :], sin_s[0:h2, :])
    nc.vector.tensor_mul(qr[h2:128, :], ps[0:h2, :], sin_s[h2:128, :])
    nc.gpsimd.tensor_add(qr[:], e1[:], qr[:])
    nc.vector.tensor_mul(out_ap, qr[:], rec[:])


_NC_CACHE = {}


def get_nc(t_seq=T, n_reps=1):
    key = (t_seq, n_reps)
    if key not in _NC_CACHE:
        _NC_CACHE[key] = build_nc(t_seq, n_reps)
    return _NC_CACHE[key]


def make_in_maps(x, cos, sin, Wq, Wk, Wv, Wo, t_seq=T):
    half = D // 2
    cosT = np.ascontiguousarray(cos.reshape(t_seq, half).T.astype(np.float32))
    sinT = np.ascontiguousarray(sin.reshape(t_seq, half).T.astype(np.float32))
    cos2 = np.concatenate([cosT, cosT], axis=0)
    sin2s = np.concatenate([sinT, -sinT], axis=0)
    wqTs, wkTs, wvTs, woTs = [], [], [], []
    for g in range(4):
        wqTs.append(np.ascontiguousarray(
            Wq[DH * g:DH * (g + 1), :].T.astype(BF16NP)))
        wkTs.append(np.ascontiguousarray(
            Wk[D * g:D * (g + 1), :].T.astype(BF16NP)))
        wvTs.append(np.ascontiguousarray(
            Wv[D * g:D * (g + 1), :].T.astype(BF16NP)))
        woTs.append(np.ascontiguousarray(
            Wo[DH * g:DH * (g + 1), :].T.astype(BF16NP)))
    xTs = [np.ascontiguousarray(x[b].T.astype(BF16NP)) for b in range(x.shape[0])]
    in_maps = []
    for c in range(N_CORES):
        b, g = c // 4, c % 4
        in_maps.append({
            "xT": xTs[b], "wqT": wqTs[g], "wkT": wkTs[g], "wvT": wvTs[g],
            "woT": woTs[g], "cos2": cos2, "sin2s": sin2s,
        })
    return in_maps


def kernel(x, cos, sin, Wq, Wk, Wv, Wo):
    x = np.asarray(x, dtype=np.float32)
    nc = get_nc(T)
    in_maps = make_in_maps(x, np.asarray(cos), np.asarray(sin),
                           np.asarray(Wq), np.asarray(Wk), np.asarray(Wv),
                           np.asarray(Wo), T)
    res = run_bass_kernel_spmd(nc, in_maps, core_ids=list(range(N_CORES)))
    outa = np.empty((B, T, C), dtype=np.float32)
    for c in range(N_CORES):
        b, g = c // 4, c % 4
        outa[b, :, DH * g:DH * (g + 1)] = res.results[c]["out"]
    return outa


# revision 18
# speedup vs baseline: 1.0110x; 1.0110x over previous
"""Trainium2 Bass kernel for CausalSelfAttention (GQA + RoPE + QK-RMSNorm).

Sharding: 8 cores = DP(2 batches) x TP(4 head-groups).
Core c handles batch b=c//4, head group g=c%4 (q heads 4g..4g+3, kv head g).
Device: QKV proj (bf16 in, fp32 accum), RoPE+RMSNorm, causal attention
(max-free softmax — QK-norm bounds |score| <= sqrt(D)), PV in bf16 with a
ones-column appended to V so row-sums come out of the same matmul, per-512-
column AllGather of the transposed attention output across each 4-core group
(overlapped with the output projection), out-proj for this group's 512 output
channels. Host concatenates the 8 per-core [T, 512] results into [B, T, C].
"""

import sys
import numpy as np

for _p in ("/opt/trn_rl_repo", "/root/.axon_site/_ro/trn_rl_repo"):
    if _p not in sys.path:
        sys.path.append(_p)

import ml_dtypes

import concourse.bass as bass
import concourse.mybir as mybir
import concourse.tile as tile
from concourse import bacc
from concourse.bass_utils import run_bass_kernel_spmd
from concourse.masks import make_identity

F32 = mybir.dt.float32
F32R = mybir.dt.float32r
BF16 = mybir.dt.bfloat16
AF = mybir.ActivationFunctionType
ALU = mybir.AluOpType
BF16NP = ml_dtypes.bfloat16

B, T, C = 2, 2048, 2048
H, KVH, D = 16, 4, 128
HLOC = H // 4          # q heads per core (TP=4)
DH = HLOC * D          # 512 output channels per core
EPS = 1.1920929e-07
N_CORES = 8

TT = 512               # t-tile (moving free dim)
# dev knobs (not used by the grading path)
CFG = {"skip_ag": False, "phases": 3, "trace_sim": False, "no_rope": False}
NKC = C // 128         # 16 contraction chunks for the projections
SM_SCALE = float(1.0 / np.sqrt(float(D)))


def build_nc(t_seq=T, n_reps=1):
    """Build the SPMD program for one core (all cores run the same program).

    n_reps > 1 replicates the whole body for on-device timing (slope method).
    """
    nc = bacc.Bacc("TRN2", target_bir_lowering=False, debug=False,
                   num_devices=N_CORES)

    xT = nc.dram_tensor("xT", [C, t_seq], BF16, kind="ExternalInput").ap()
    wqT = nc.dram_tensor("wqT", [C, DH], BF16, kind="ExternalInput").ap()
    wkT = nc.dram_tensor("wkT", [C, D], BF16, kind="ExternalInput").ap()
    wvT = nc.dram_tensor("wvT", [C, D], BF16, kind="ExternalInput").ap()
    woT = nc.dram_tensor("woT", [C, DH], BF16, kind="ExternalInput").ap()
    cos2 = nc.dram_tensor("cos2", [D, t_seq], F32, kind="ExternalInput").ap()
    sin2s = nc.dram_tensor("sin2s", [D, t_seq], F32, kind="ExternalInput").ap()
    out = nc.dram_tensor("out", [t_seq, DH], F32, kind="ExternalOutput").ap()

    groups = [[0, 1, 2, 3], [4, 5, 6, 7]]

    with tile.TileContext(nc, trace_sim=CFG["trace_sim"]) as tc:
        for _ in range(n_reps):
            build_body(tc, nc, xT, wqT, wkT, wvT, woT, cos2, sin2s, out,
                       groups, t_seq)
    nc.compile()
    return nc


def build_body(tc, nc, xT, wqT, wkT, wvT, woT, cos2, sin2s, out,
               groups, t_seq):
    from contextlib import ExitStack

    tt = TT
    nt = t_seq // tt       # t tiles
    nkb = t_seq // 128     # key blocks

    ctx = ExitStack()
    with ctx:
        # ---------- persistent pools ----------
        const_pool = ctx.enter_context(tc.tile_pool(name="const", bufs=1))
        qk_pool = ctx.enter_context(tc.tile_pool(name="qk", bufs=1))
        yv_pool = ctx.enter_context(tc.tile_pool(name="yv", bufs=1))
        dram = ctx.enter_context(tc.tile_pool(name="dram", bufs=1, space="DRAM"))

        ident = const_pool.tile([128, 128], F32, name="ident")
        make_identity(nc, ident[:])
        ident_bf = const_pool.tile([128, 128], BF16, name="ident_bf")
        nc.scalar.activation(ident_bf[:], ident[:], AF.Copy)
        ones_f = const_pool.tile([128, 1], F32, name="ones_f")
        nc.gpsimd.memset(ones_f[:], 1.0)
        ones_col = const_pool.tile([128, 1], F32R, name="ones_col")
        nc.scalar.activation(ones_col[:], ones_f[:], AF.Copy)
        ones_row = const_pool.tile([1, 128], F32R, name="ones_row")
        nc.scalar.activation(ones_row[:], ones_f[0:1, :].to_broadcast([1, 128]),
                             AF.Copy)
        eps_t = const_pool.tile([1, 1], F32, name="eps_t")
        nc.gpsimd.memset(eps_t[:], EPS)
        smsc_f = const_pool.tile([1, 1], F32, name="smsc_f")
        nc.gpsimd.memset(smsc_f[:], SM_SCALE)

        # qT/kT normalized+roped, [D, t_seq] per head (bf16)
        qTn = [qk_pool.tile([128, t_seq], BF16, name=f"qTn{h}") for h in range(HLOC)]
        kTn = qk_pool.tile([128, t_seq], BF16, name="kTn")
        # per-key-block exp scales: [128, nkb] = SM_SCALE / rms(k)[tk]
        rks = yv_pool.tile([128, nkb], F32, name="rks")
        # v_aug: per key block, [128 tk, 129] bf16 (col 128 = 1.0)
        v_aug = [yv_pool.tile([128, 129], BF16, name=f"vaug{j}") for j in range(nkb)]
        # attention output transposed: HLOC head-chunks of [128 c, t_seq]
        yT = [yv_pool.tile([128, t_seq], BF16, name=f"yT{h}") for h in range(HLOC)]

        # ================= phase 1: QKV projections =================
        with (
            tc.tile_pool(name="p1x", bufs=1) as p1x,
            tc.tile_pool(name="p1w", bufs=1) as p1w,
            tc.tile_pool(name="p1t", bufs=2) as p1t,
            tc.tile_pool(name="p1ps", bufs=3, space="PSUM") as p1ps,
            tc.tile_pool(name="p1sw", bufs=2, space="PSUM") as p1sw,
            tc.tile_pool(name="p1ss", bufs=1, space="PSUM") as p1ss,
        ):
            # weights (transposed, c-major) stay resident for phase 1.
            # Interleave x-chunk-0 and weight DMAs per c so the first
            # projection matmul can start after ~0.25 MB instead of ~5 MB.
            wq_sb = [p1w.tile([128, DH], BF16, name=f"wq{c}") for c in range(NKC)]
            wk_sb = [p1w.tile([128, D], BF16, name=f"wk{c}") for c in range(NKC)]
            wv_sb = [p1w.tile([128, D], BF16, name=f"wv{c}") for c in range(NKC)]
            cos_sb = p1w.tile([128, t_seq], F32, name="cos_sb")
            sin_sb = p1w.tile([128, t_seq], F32, name="sin_sb")
            nc.gpsimd.dma_start(cos_sb[:], cos2[:])
            nc.gpsimd.dma_start(sin_sb[:], sin2s[:])
            # x resident for all of phase 1: 16 tiles of [128, t_seq] bf16,
            # one big 4KB/descriptor DMA each, spread over 4 queues
            x_sb = [p1x.tile([128, t_seq], BF16, name=f"x{c}")
                    for c in range(NKC)]
            dmaq = [nc.sync, nc.scalar, nc.gpsimd]
            for c in range(NKC):
                dmaq[c % 3].dma_start(x_sb[c][:], xT[128 * c:128 * (c + 1), :])
                nc.sync.dma_start(wq_sb[c][:], wqT[128 * c:128 * (c + 1), :])
            for c in range(NKC):
                nc.sync.dma_start(wk_sb[c][:], wkT[128 * c:128 * (c + 1), :])
                nc.sync.dma_start(wv_sb[c][:], wvT[128 * c:128 * (c + 1), :])
            vT = p1w.tile([128, t_seq], BF16, name="vT")

            # Software-pipelined unit stream: per i-tile, units q0..q3, k, v.
            # The rope-path PE matmuls (sumsq `ss`, broadcast `rb`, the rk/v
            # transposes) are emitted 1-2 units late so the in-order PE queue
            # never waits on an ACT/DVE round-trip: part0 at unit emission,
            # part1 (ss) one unit later, part2 (rb + finish) two units later.
            units = []
            for i in range(nt):
                for h in range(HLOC):
                    units.append(("q", i, h))
                units.append(("k", i, 0))
                units.append(("v", i, 0))

            state = {}

            def emit_proj_part0(u):
                kind, i, h = units[u]
                ts = slice(i * tt, (i + 1) * tt)
                xt = [x_sb[c][:, ts] for c in range(NKC)]
                ps = p1ps.tile([128, tt], F32, name="qkv_ps")
                if kind == "q":
                    w = [wq_sb[c][:, 128 * h:128 * (h + 1)] for c in range(NKC)]
                elif kind == "k":
                    w = [wk_sb[c][:] for c in range(NKC)]
                else:
                    w = [wv_sb[c][:] for c in range(NKC)]
                for c in range(NKC):
                    nc.tensor.matmul(ps[:], w[c], xt[c],
                                     start=(c == 0), stop=(c == NKC - 1))
                st = {"ps": ps, "i": i, "h": h, "kind": kind}
                if kind == "v":
                    nc.scalar.activation(vT[:, ts], ps[:], AF.Copy)
                elif CFG["no_rope"]:
                    dst = qTn[h][:, ts] if kind == "q" else kTn[:, ts]
                    nc.scalar.activation(dst, ps[:], AF.Copy)
                else:
                    # part0 of rope: square (ACT), rotation muls (DVE), add
                    sq = p1t.tile([128, tt], F32R, name="sq", tag="sq",
                                  bufs=3)
                    nc.scalar.activation(sq[:], ps[:], AF.Square)
                    e1 = p1t.tile([128, tt], F32, name="e1", tag="e1",
                                  bufs=2)
                    nc.vector.tensor_mul(e1[:], ps[:], cos_sb[:, ts])
                    qr = p1t.tile([128, tt], F32, name="qr", tag="qr",
                                  bufs=4)
                    nc.vector.tensor_mul(qr[0:64, :], ps[64:128, :],
                                         sin_sb[0:64, ts])
                    nc.vector.tensor_mul(qr[64:128, :], ps[0:64, :],
                                         sin_sb[64:128, ts])
                    if kind == "k":
                        # k is left unnormalized; rotation goes straight out
                        nc.gpsimd.tensor_add(kTn[:, ts], e1[:], qr[:])
                    else:
                        nc.gpsimd.tensor_add(qr[:], e1[:], qr[:])
                    st["sq"] = sq
                    st["qr"] = qr
                state[u] = st

            def emit_part1(u):
                st = state[u]
                if CFG["no_rope"] or st["kind"] == "v":
                    return
                # sumsq over d via PE, then sqrt on ACT
                ss = p1ss.tile([1, tt], F32, name="ss_ps", tag="ss", bufs=1)
                nc.tensor.matmul(ss[:], ones_col[:], st["sq"][:])
                sd = p1t.tile([1, tt], F32R, name="sd", tag="sd", bufs=2)
                nc.scalar.activation(sd[:], ss[:], AF.Sqrt, scale=1.0 / 128.0,
                                     bias=eps_t[:])
                st["sd"] = sd

            def emit_part2(u):
                st = state.pop(u)
                kind, i = st["kind"], st["i"]
                ts = slice(i * tt, (i + 1) * tt)
                if kind == "v":
                    # transposes to [tk, d] + bf16 v_aug (vT written part0)
                    for j in range(4 * i, min(4 * i + 4, nkb)):
                        tp = p1sw.tile([128, 128], BF16, name="v_tp",
                                       tag="sw_ps", bufs=2)
                        nc.tensor.matmul(tp[:], vT[:, 128 * j:128 * (j + 1)],
                                         ident_bf[:], is_transpose=True)
                        nc.gpsimd.memset(v_aug[j][:, 128:129], 1.0)
                        nc.scalar.activation(v_aug[j][:, 0:128], tp[:], AF.Copy)
                    return
                if CFG["no_rope"]:
                    return
                if kind == "q":
                    rb = p1ss.tile([128, tt], F32, name="rb_ps", tag="rb",
                                   bufs=1)
                    nc.tensor.matmul(rb[:], ones_row[:], st["sd"][:])
                    rec = p1t.tile([128, tt], F32, name="rec", tag="rec",
                                   bufs=2)
                    nc.vector.reciprocal(rec[:], rb[:])
                    nc.vector.tensor_mul(qTn[st["h"]][:, ts], st["qr"][:],
                                         rec[:])
                else:  # k: per-block exp scales SM_SCALE/rms via PE transpose
                    rr = p1t.tile([1, tt], F32, name="rr", tag="rr", bufs=2)
                    nc.vector.reciprocal(rr[:], st["sd"][:].bitcast(F32))
                    rkp = p1ss.tile([128, 4], F32, name="rk_ps", tag="rk",
                                    bufs=1)
                    for b in range(4):
                        nc.tensor.matmul(rkp[:, b:b + 1],
                                         rr[0:1, 128 * b:128 * (b + 1)],
                                         smsc_f[:])
                    nc.vector.tensor_copy(rks[:, 4 * i:4 * i + 4], rkp[:])

            for u in range(len(units)):
                emit_proj_part0(u)
                if u >= 1:
                    emit_part1(u - 1)
                if u >= 2:
                    emit_part2(u - 2)
            nu = len(units)
            emit_part1(nu - 1)
            emit_part2(nu - 2)
            emit_part2(nu - 1)

        # ================= phase 2+3 shared SBUF =================
        with tc.tile_pool(name="p2m", bufs=1) as p2m:
            masks = []
            for r in range(4):
                m = p2m.tile([128, tt], BF16, name=f"mask{r}")
                nc.gpsimd.memset(m[:], 1.0)
                nc.gpsimd.affine_select(
                    out=m[:], in_=m[:], compare_op=ALU.is_ge, fill=0.0,
                    base=-128 * r, pattern=[[1, tt]], channel_multiplier=-1)
                masks.append(m)

            # wo tiles (DMA overlaps with attention)
            wo_sb = [p2m.tile([128, DH], BF16, name=f"wo{c}") for c in range(NKC)]
            for c in range(NKC):
                nc.gpsimd.dma_start(wo_sb[c][:], woT[128 * c:128 * (c + 1), :])

            # last tile's AG is split into two head-pair halves so the
            # final out-proj can start before the last heads finish
            ag_in = [dram.tile([DH, tt], BF16, name=f"ag_in{i}")
                     for i in range(nt - 1)]
            ag_out = [dram.tile([4 * DH, tt], BF16, name=f"ag_out{i}")
                      for i in range(nt - 1)]
            ag_in3 = [dram.tile([2 * 128, tt], BF16, name=f"ag_in3{p}")
                      for p in range(2)]
            ag_out3 = [dram.tile([4 * 2 * 128, tt], BF16, name=f"ag_out3{p}")
                       for p in range(2)]

            # ---------- phase 2: causal attention ----------
            with (
                tc.tile_pool(name="p2pt", bufs=6) as p2pt,
                tc.tile_pool(name="p2on", bufs=6) as p2on,
                tc.tile_pool(name="p2st", bufs=2, space="PSUM") as p2st,
                tc.tile_pool(name="p2o", bufs=1, space="PSUM") as p2o,
                tc.tile_pool(name="p2tp", bufs=1, space="PSUM") as p2tp,
                tc.tile_pool(name="p3y", bufs=1) as p3y,
                tc.tile_pool(name="p3t", bufs=6) as p3t,
            ):
                for i in range(nt if CFG["phases"] >= 2 else 0):
                    nj = min(4 * i + 4, nkb)
                    for h in range(HLOC):
                        # token-blocks t=0,1 and t=2,3 pair up in one PSUM
                        # bank each, as a single accumulation group per bank
                        o01 = p2o.tile([128, 258], F32, name="o01", tag="o01",
                                       bufs=2)
                        o23 = p2o.tile([128, 258], F32, name="o23", tag="o23",
                                       bufs=2)
                        o_of = {0: (o01, 0), 1: (o01, 129),
                                2: (o23, 0), 3: (o23, 129)}
                        for j in range(nj):
                            r = j - 4 * i
                            # diagonal blocks r=1,2,3: the first 128*r score
                            # columns are fully masked — skip them
                            off = 128 * r if r > 0 else 0
                            npr = tt - off
                            st = p2st.tile([128, tt], F32, name="st_ps")
                            nc.tensor.matmul(
                                st[:, 0:npr], kTn[:, 128 * j:128 * (j + 1)],
                                qTn[h][:, i * tt + off:(i + 1) * tt])
                            pt = p2pt.tile([128, tt], BF16, name="pt")
                            nc.scalar.activation(pt[:, 0:npr], st[:, 0:npr],
                                                 AF.Exp,
                                                 scale=rks[:, j:j + 1])
                            if r >= 0:
                                nc.vector.tensor_mul(pt[:, 0:npr], pt[:, 0:npr],
                                                     masks[r][:, off:tt])
                            for t in range(4):
                                if j <= 4 * i + t:
                                    ot_ps, oc = o_of[t]
                                    nc.tensor.matmul(
                                        ot_ps[:, oc:oc + 129],
                                        pt[:, 128 * t - off:128 * (t + 1) - off],
                                        v_aug[j][:],
                                        start=(j == 0 and t % 2 == 0),
                                        stop=(j == 4 * i + t and t % 2 == 1))
                        for t in range(4):
                            ot_ps, oc = o_of[t]
                            rec = p2on.tile([128, 1], F32, name="rec")
                            nc.vector.reciprocal(
                                rec[:], ot_ps[:, oc + 128:oc + 129])
                            o_n = p2on.tile([128, 128], BF16, name="o_n")
                            nc.vector.tensor_scalar_mul(
                                o_n[:], ot_ps[:, oc:oc + 128], rec[:])
                            tp = p2tp.tile([128, 128], BF16, name="o_tp")
                            nc.tensor.matmul(tp[:], o_n[:], ident_bf[:],
                                             is_transpose=True)
                            nc.vector.tensor_copy(
                                yT[h][:, i * tt + 128 * t:i * tt + 128 * (t + 1)],
                                tp[:])
                        if i == nt - 1:
                            # eager per-head staging; AG per head-pair
                            pair, ha = h // 2, h % 2
                            nc.sync.dma_start(
                                ag_in3[pair][128 * ha:128 * (ha + 1), :],
                                yT[h][:, i * tt:(i + 1) * tt])
                            if ha == 1 and not CFG["skip_ag"]:
                                nc.gpsimd.collective_compute(
                                    "AllGather", ALU.bypass,
                                    replica_groups=groups,
                                    ins=[ag_in3[pair][:]],
                                    outs=[ag_out3[pair][:]])
                    if i < nt - 1:
                        # yT chunk i complete for all heads -> stage + AllGather
                        for h in range(HLOC):
                            nc.sync.dma_start(ag_in[i][128 * h:128 * (h + 1), :],
                                              yT[h][:, i * tt:(i + 1) * tt])
                        if not CFG["skip_ag"]:
                            nc.gpsimd.collective_compute(
                                "AllGather", ALU.bypass, replica_groups=groups,
                                ins=[ag_in[i][:]], outs=[ag_out[i][:]])

                # ---------- phase 3: output projection ----------
                for i in range(nt if CFG["phases"] >= 3 else 0):
                    if i < nt - 1:
                        order = list(range(NKC))
                        srcs = [ag_out[i][128 * c:128 * (c + 1), :]
                                for c in order]
                    else:
                        # wave A (c%4 in 0,1 — from ag_out3[0]) first, so
                        # the final accumulation can start before AG3b lands
                        order = ([c for c in range(NKC) if c % 4 < 2]
                                 + [c for c in range(NKC) if c % 4 >= 2])
                        srcs = []
                        for c in order:
                            pair, hl = (0, c % 4) if c % 4 < 2 else (1, c % 4 - 2)
                            base = 256 * (c // 4) + 128 * hl
                            srcs.append(ag_out3[pair][base:base + 128, :])
                    yf = {}
                    for c, src in zip(order, srcs):
                        yc = p3y.tile([128, tt], BF16, name=f"yf{c}", tag="yf",
                                      bufs=2 * NKC)
                        nc.sync.dma_start(yc[:], src)
                        yf[c] = yc
                    for t in range(4):
                        ps = p2st.tile([128, DH], F32, name="out_ps", tag="cp",
                                       bufs=1)
                        for k, c in enumerate(order):
                            nc.tensor.matmul(
                                ps[:], yf[c][:, 128 * t:128 * (t + 1)],
                                wo_sb[c][:],
                                start=(k == 0), stop=(k == NKC - 1))
                        ot = p3t.tile([128, DH], F32, name="ot")
                        nc.vector.tensor_copy(ot[:], ps[:])
                        nc.sync.dma_start(
                            out[i * tt + 128 * t:i * tt + 128 * (t + 1), :],
                            ot[:])


def rope_only(nc, tmp_pool, sw_pool, ss_pool, ps, cos, sin_s, out_ap,
              ones_col, eps_t, smsc, rks_blocks):
    """RoPE for k; writes roped (unnormalized) k to out_ap and the per-tk
    exp scales SM_SCALE/rms into rks_blocks ([128,1] each, via PE transpose
    of the [1, tt] reciprocal-rms row).

    RoPE is orthogonal per (t, d-pair), so rms(rope(x)) = rms(x): the rms
    path reads the pre-rope PSUM directly and runs concurrently with the
    rotation. The half-swap is two 64-partition DVE muls at partition
    offsets (no PE swap matmul, no staging copy).
    """
    ttl = ps.shape[-1]
    h2 = 64
    # rms path (from pre-rope ps)
    sq = tmp_pool.tile([128, ttl], F32R, name="sq", tag="sq")
    nc.scalar.activation(sq[:], ps[:], AF.Square)
    ss = ss_pool.tile([1, ttl], F32, name="ss_ps", tag="ss")
    nc.tensor.matmul(ss[:], ones_col[:], sq[:])
    sd = tmp_pool.tile([1, ttl], F32, name="sd", tag="sd")
    nc.scalar.activation(sd[:], ss[:], AF.Sqrt, scale=1.0 / 128.0,
                         bias=eps_t[:])
    rr = tmp_pool.tile([1, ttl], F32, name="rr", tag="rr")
    nc.vector.reciprocal(rr[:], sd[:])
    for b, rk in enumerate(rks_blocks):
        rkp = ss_pool.tile([128, 1], F32, name="rk_ps", tag="rb")
        nc.tensor.matmul(rkp[:], rr[0:1, 128 * b:128 * (b + 1)], smsc[:])
        nc.vector.tensor_copy(rk[:], rkp[:])
    # rotation path
    e1 = tmp_pool.tile([128, ttl], F32, name="e1", tag="e1")
    nc.vector.tensor_mul(e1[:], ps[:], cos)
    qr = tmp_pool.tile([128, ttl], F32, name="qr", tag="qr")
    nc.vector.tensor_mul(qr[0:h2, :], ps[h2:128, :], sin_s[0:h2, :])
    nc.vector.tensor_mul(qr[h2:128, :], ps[0:h2, :], sin_s[h2:128, :])
    nc.gpsimd.tensor_add(out_ap, e1[:], qr[:])


def rope_norm(nc, tmp_pool, sw_pool, ss_pool, ps, cos, sin_s, out_ap,
              ones_col, ones_row, eps_t):
    """RoPE + RMS-norm. ps: [128 d, tt] PSUM (pre-rope head), out_ap: SBUF.

    cos is [cos; cos] (rows duplicated), sin_s is [sin; -sin]. Rotation
    preserves the per-t norm, so the rms factor is computed from the
    pre-rope PSUM concurrently with the rotation, then applied once at
    the end: out = (ps*cos + swap(ps)*sin_s) * rsqrt(mean(ps^2)+eps).
    """
    ttl = ps.shape[-1]
    h2 = 64
    # rms path: sumsq over d via PE, rsqrt on ACT, bcast to 128 parts via PE
    sq = tmp_pool.tile([128, ttl], F32R, name="sq", tag="sq")
    nc.scalar.activation(sq[:], ps[:], AF.Square)
    ss = ss_pool.tile([1, ttl], F32, name="ss_ps", tag="ss")
    nc.tensor.matmul(ss[:], ones_col[:], sq[:])
    sd = tmp_pool.tile([1, ttl], F32R, name="sd", tag="sd")
    nc.scalar.activation(sd[:], ss[:], AF.Sqrt, scale=1.0 / 128.0,
                         bias=eps_t[:])
    rb = ss_pool.tile([128, ttl], F32, name="rb_ps", tag="rb")
    nc.tensor.matmul(rb[:], ones_row[:], sd[:])
    rec = tmp_pool.tile([128, ttl], F32, name="rec", tag="rec")
    nc.vector.reciprocal(rec[:], rb[:])
    # rotation path (reads PSUM directly; half-swap via partition offsets)
    e1 = tmp_pool.tile([128, ttl], F32, name="e1", tag="e1")
    nc.vector.tensor_mul(e1[:], ps[:], cos)
    qr = tmp_pool.tile([128, ttl], F32, name="qr", tag="qr")
    nc.vector.tensor_mul(qr[0:h2, :], ps[h2:128, :], sin_s[0:h2, :])
    nc.vector.tensor_mul(qr[h2:128, :], ps[0:h2, :], sin_s[h2:128, :])
    nc.gpsimd.tensor_add(qr[:], e1[:], qr[:])
    nc.vector.tensor_mul(out_ap, qr[:], rec[:])


_NC_CACHE = {}


def get_nc(t_seq=T, n_reps=1):
    key = (t_seq, n_reps)
    if key not in _NC_CACHE:
        _NC_CACHE[key] = build_nc(t_seq, n_reps)
    return _NC_CACHE[key]


def make_in_maps(x, cos, sin, Wq, Wk, Wv, Wo, t_seq=T):
    half = D // 2
    cosT = np.ascontiguousarray(cos.reshape(t_seq, half).T.astype(np.float32))
    sinT = np.ascontiguousarray(sin.reshape(t_seq, half).T.astype(np.float32))
    cos2 = np.concatenate([cosT, cosT], axis=0)
    sin2s = np.concatenate([sinT, -sinT], axis=0)
    wqTs, wkTs, wvTs, woTs = [], [], [], []
    for g in range(4):
        wqTs.append(np.ascontiguousarray(
            Wq[DH * g:DH * (g + 1), :].T.astype(BF16NP)))
        wkTs.append(np.ascontiguousarray(
            Wk[D * g:D * (g + 1), :].T.astype(BF16NP)))
        wvTs.append(np.ascontiguousarray(
            Wv[D * g:D * (g + 1), :].T.astype(BF16NP)))
        woTs.append(np.ascontiguousarray(
            Wo[DH * g:DH * (g + 1), :].T.astype(BF16NP)))
    xTs = [np.ascontiguousarray(x[b].T.astype(BF16NP)) for b in range(x.shape[0])]
    in_maps = []
    for c in range(N_CORES):
        b, g = c // 4, c % 4
        in_maps.append({
            "xT": xTs[b], "wqT": wqTs[g], "wkT": wkTs[g], "wvT": wvTs[g],
            "woT": woTs[g], "cos2": cos2, "sin2s": sin2s,
        })
    return in_maps


def kernel(x, cos, sin, Wq, Wk, Wv, Wo):
    x = np.asarray(x, dtype=np.float32)
    nc = get_nc(T)
    in_maps = make_in_maps(x, np.asarray(cos), np.asarray(sin),
                           np.asarray(Wq), np.asarray(Wk), np.asarray(Wv),
                           np.asarray(Wo), T)
    res = run_bass_kernel_spmd(nc, in_maps, core_ids=list(range(N_CORES)))
    outa = np.empty((B, T, C), dtype=np.float32)
    for c in range(N_CORES):
        b, g = c // 4, c % 4
        outa[b, :, DH * g:DH * (g + 1)] = res.results[c]["out"]
    return outa


# revision 27
# speedup vs baseline: 17.1515x; 16.9655x over previous
"""Trainium2 Bass kernel for CausalSelfAttention (GQA + RoPE + QK-RMSNorm).

Sharding: 8 cores = DP(2 batches) x TP(4 head-groups).
Core c handles batch b=c//4, head group g=c%4 (q heads 4g..4g+3, kv head g).
Device: QKV proj (bf16 in, fp32 accum), RoPE+RMSNorm, causal attention
(max-free softmax — QK-norm bounds |score| <= sqrt(D)), PV in bf16 with a
ones-column appended to V so row-sums come out of the same matmul, per-512-
column AllGather of the transposed attention output across each 4-core group
(overlapped with the output projection), out-proj for this group's 512 output
channels. Host concatenates the 8 per-core [T, 512] results into [B, T, C].
"""

import sys
import numpy as np

for _p in ("/opt/trn_rl_repo", "/root/.axon_site/_ro/trn_rl_repo"):
    if _p not in sys.path:
        sys.path.append(_p)

import ml_dtypes

import concourse.bass as bass
import concourse.mybir as mybir
import concourse.tile as tile
from concourse import bacc
from concourse.bass_utils import run_bass_kernel_spmd
from concourse.masks import make_identity

F32 = mybir.dt.float32
F32R = mybir.dt.float32r
BF16 = mybir.dt.bfloat16
AF = mybir.ActivationFunctionType
ALU = mybir.AluOpType
BF16NP = ml_dtypes.bfloat16

B, T, C = 2, 2048, 2048
H, KVH, D = 16, 4, 128
HLOC = H // 4          # q heads per core (TP=4)
DH = HLOC * D          # 512 output channels per core
EPS = 1.1920929e-07
N_CORES = 8

TT = 512               # t-tile (moving free dim)
# dev knobs (not used by the grading path)
CFG = {"skip_ag": False, "phases": 3, "trace_sim": False, "no_rope": False}
NKC = C // 128         # 16 contraction chunks for the projections
SM_SCALE = float(1.0 / np.sqrt(float(D)))


def build_nc(t_seq=T, n_reps=1):
    """Build the SPMD program for one core (all cores run the same program).

    n_reps > 1 replicates the whole body for on-device timing (slope method).
    """
    nc = bacc.Bacc("TRN2", target_bir_lowering=False, debug=False,
                   num_devices=N_CORES)

    xT = nc.dram_tensor("xT", [C, t_seq], BF16, kind="ExternalInput").ap()
    wqT = nc.dram_tensor("wqT", [C, DH], BF16, kind="ExternalInput").ap()
    wkT = nc.dram_tensor("wkT", [C, D], BF16, kind="ExternalInput").ap()
    wvT = nc.dram_tensor("wvT", [C, D], BF16, kind="ExternalInput").ap()
    woT = nc.dram_tensor("woT", [C, DH], BF16, kind="ExternalInput").ap()
    cos2 = nc.dram_tensor("cos2", [D, t_seq], F32, kind="ExternalInput").ap()
    sin2s = nc.dram_tensor("sin2s", [D, t_seq], F32, kind="ExternalInput").ap()
    out = nc.dram_tensor("out", [t_seq, DH], F32, kind="ExternalOutput").ap()

    groups = [[0, 1, 2, 3], [4, 5, 6, 7]]

    with tile.TileContext(nc, trace_sim=CFG["trace_sim"]) as tc:
        for _ in range(n_reps):
            build_body(tc, nc, xT, wqT, wkT, wvT, woT, cos2, sin2s, out,
                       groups, t_seq)
    nc.compile()
    return nc


def build_body(tc, nc, xT, wqT, wkT, wvT, woT, cos2, sin2s, out,
               groups, t_seq):
    from contextlib import ExitStack

    tt = TT
    nt = t_seq // tt       # t tiles
    nkb = t_seq // 128     # key blocks

    ctx = ExitStack()
    with ctx:
        # ---------- persistent pools ----------
        const_pool = ctx.enter_context(tc.tile_pool(name="const", bufs=1))
        qk_pool = ctx.enter_context(tc.tile_pool(name="qk", bufs=1))
        yv_pool = ctx.enter_context(tc.tile_pool(name="yv", bufs=1))
        dram = ctx.enter_context(tc.tile_pool(name="dram", bufs=1, space="DRAM"))

        ident = const_pool.tile([128, 128], F32, name="ident")
        make_identity(nc, ident[:])
        ident_bf = const_pool.tile([128, 128], BF16, name="ident_bf")
        nc.scalar.activation(ident_bf[:], ident[:], AF.Copy)
        ones_f = const_pool.tile([128, 1], F32, name="ones_f")
        nc.gpsimd.memset(ones_f[:], 1.0)
        ones_col = const_pool.tile([128, 1], F32R, name="ones_col")
        nc.scalar.activation(ones_col[:], ones_f[:], AF.Copy)
        ones_row = const_pool.tile([1, 128], F32R, name="ones_row")
        nc.scalar.activation(ones_row[:], ones_f[0:1, :].to_broadcast([1, 128]),
                             AF.Copy)
        eps_t = const_pool.tile([1, 1], F32, name="eps_t")
        nc.gpsimd.memset(eps_t[:], EPS)
        smsc_f = const_pool.tile([1, 1], F32, name="smsc_f")
        nc.gpsimd.memset(smsc_f[:], SM_SCALE)

        # qT/kT normalized+roped, [D, t_seq] per head (bf16)
        qTn = [qk_pool.tile([128, t_seq], BF16, name=f"qTn{h}") for h in range(HLOC)]
        kTn = qk_pool.tile([128, t_seq], BF16, name="kTn")
        # per-key-block exp scales: [128, nkb] = SM_SCALE / rms(k)[tk]
        rks = yv_pool.tile([128, nkb], F32, name="rks")
        # v_aug: per key block, [128 tk, 129] bf16 (col 128 = 1.0)
        v_aug = [yv_pool.tile([128, 129], BF16, name=f"vaug{j}") for j in range(nkb)]
        # attention output transposed: HLOC head-chunks of [128 c, t_seq]
        yT = [yv_pool.tile([128, t_seq], BF16, name=f"yT{h}") for h in range(HLOC)]

        # ================= phase 1: QKV projections =================
        with (
            tc.tile_pool(name="p1x", bufs=1) as p1x,
            tc.tile_pool(name="p1w", bufs=1) as p1w,
            tc.tile_pool(name="p1t", bufs=2) as p1t,
            tc.tile_pool(name="p1ps", bufs=3, space="PSUM") as p1ps,
            tc.tile_pool(name="p1sw", bufs=2, space="PSUM") as p1sw,
            tc.tile_pool(name="p1ss", bufs=1, space="PSUM") as p1ss,
        ):
            # weights (transposed, c-major) stay resident for phase 1.
            # Interleave x-chunk-0 and weight DMAs per c so the first
            # projection matmul can start after ~0.25 MB instead of ~5 MB.
            wq_sb = [p1w.tile([128, DH], BF16, name=f"wq{c}") for c in range(NKC)]
            wk_sb = [p1w.tile([128, D], BF16, name=f"wk{c}") for c in range(NKC)]
            wv_sb = [p1w.tile([128, D], BF16, name=f"wv{c}") for c in range(NKC)]
            cos_sb = p1w.tile([128, t_seq], F32, name="cos_sb")
            sin_sb = p1w.tile([128, t_seq], F32, name="sin_sb")
            nc.gpsimd.dma_start(cos_sb[:], cos2[:])
            nc.gpsimd.dma_start(sin_sb[:], sin2s[:])
            # x resident for all of phase 1: 16 tiles of [128, t_seq] bf16,
            # one big 4KB/descriptor DMA each, spread over 4 queues
            x_sb = [p1x.tile([128, t_seq], BF16, name=f"x{c}")
                    for c in range(NKC)]
            dmaq = [nc.sync, nc.scalar, nc.gpsimd]
            for c in range(NKC):
                dmaq[c % 3].dma_start(x_sb[c][:], xT[128 * c:128 * (c + 1), :])
                nc.sync.dma_start(wq_sb[c][:], wqT[128 * c:128 * (c + 1), :])
            for c in range(NKC):
                nc.sync.dma_start(wk_sb[c][:], wkT[128 * c:128 * (c + 1), :])
                nc.sync.dma_start(wv_sb[c][:], wvT[128 * c:128 * (c + 1), :])
            vT = p1w.tile([128, t_seq], BF16, name="vT")

            # Software-pipelined unit stream: per i-tile, units q0..q3, k, v.
            # The rope-path PE matmuls (sumsq `ss`, broadcast `rb`, the rk/v
            # transposes) are emitted 1-2 units late so the in-order PE queue
            # never waits on an ACT/DVE round-trip: part0 at unit emission,
            # part1 (ss) one unit later, part2 (rb + finish) two units later.
            units = []
            for i in range(nt):
                for h in range(HLOC):
                    units.append(("q", i, h))
                units.append(("k", i, 0))
                units.append(("v", i, 0))

            state = {}

            def emit_proj_part0(u):
                kind, i, h = units[u]
                ts = slice(i * tt, (i + 1) * tt)
                xt = [x_sb[c][:, ts] for c in range(NKC)]
                ps = p1ps.tile([128, tt], F32, name="qkv_ps")
                if kind == "q":
                    w = [wq_sb[c][:, 128 * h:128 * (h + 1)] for c in range(NKC)]
                elif kind == "k":
                    w = [wk_sb[c][:] for c in range(NKC)]
                else:
                    w = [wv_sb[c][:] for c in range(NKC)]
                for c in range(NKC):
                    nc.tensor.matmul(ps[:], w[c], xt[c],
                                     start=(c == 0), stop=(c == NKC - 1))
                st = {"ps": ps, "i": i, "h": h, "kind": kind}
                if kind == "v":
                    nc.scalar.activation(vT[:, ts], ps[:], AF.Copy)
                elif CFG["no_rope"]:
                    dst = qTn[h][:, ts] if kind == "q" else kTn[:, ts]
                    nc.scalar.activation(dst, ps[:], AF.Copy)
                else:
                    # part0 of rope: square (ACT), rotation muls (DVE), add
                    sq = p1t.tile([128, tt], F32R, name="sq", tag="sq",
                                  bufs=3)
                    nc.scalar.activation(sq[:], ps[:], AF.Square)
                    e1 = p1t.tile([128, tt], F32, name="e1", tag="e1",
                                  bufs=2)
                    nc.vector.tensor_mul(e1[:], ps[:], cos_sb[:, ts])
                    qr = p1t.tile([128, tt], F32, name="qr", tag="qr",
                                  bufs=4)
                    nc.vector.tensor_mul(qr[0:64, :], ps[64:128, :],
                                         sin_sb[0:64, ts])
                    nc.vector.tensor_mul(qr[64:128, :], ps[0:64, :],
                                         sin_sb[64:128, ts])
                    if kind == "k" or CFG.get("no_norm"):
                        # k is left unnormalized; rotation goes straight out
                        dst = kTn[:, ts] if kind == "k" else qTn[h][:, ts]
                        nc.gpsimd.tensor_add(dst, e1[:], qr[:])
                    else:
                        nc.gpsimd.tensor_add(qr[:], e1[:], qr[:])
                    st["sq"] = sq
                    st["qr"] = qr
                state[u] = st

            def emit_part1(u):
                st = state[u]
                if CFG["no_rope"] or CFG.get("no_norm") or st["kind"] == "v":
                    return
                # sumsq over d via PE, then sqrt on ACT
                ss = p1ss.tile([1, tt], F32, name="ss_ps", tag="ss", bufs=1)
                nc.tensor.matmul(ss[:], ones_col[:], st["sq"][:])
                sd = p1t.tile([1, tt], F32R, name="sd", tag="sd", bufs=2)
                nc.scalar.activation(sd[:], ss[:], AF.Sqrt, scale=1.0 / 128.0,
                                     bias=eps_t[:])
                st["sd"] = sd
                if st["kind"] == "k":
                    # reciprocal a stage early so the rk transposes (PE)
                    # never wait on the DVE backlog
                    rr = p1t.tile([1, tt], F32, name="rr", tag="rr", bufs=2)
                    nc.vector.reciprocal(rr[:], sd[:].bitcast(F32))
                    st["rr"] = rr

            def emit_part2(u):
                st = state.pop(u)
                kind, i = st["kind"], st["i"]
                ts = slice(i * tt, (i + 1) * tt)
                if kind == "v":
                    # transposes to [tk, d] + bf16 v_aug (vT written part0)
                    for j in range(4 * i, min(4 * i + 4, nkb)):
                        tp = p1sw.tile([128, 128], BF16, name="v_tp",
                                       tag="sw_ps", bufs=2)
                        nc.tensor.matmul(tp[:], vT[:, 128 * j:128 * (j + 1)],
                                         ident_bf[:], is_transpose=True)
                        nc.gpsimd.memset(v_aug[j][:, 128:129], 1.0)
                        nc.scalar.activation(v_aug[j][:, 0:128], tp[:], AF.Copy)
                    return
                if CFG["no_rope"] or CFG.get("no_norm"):
                    return
                if kind == "q":
                    rb = p1ss.tile([128, tt], F32, name="rb_ps", tag="rb",
                                   bufs=1)
                    nc.tensor.matmul(rb[:], ones_row[:], st["sd"][:])
                    rec = p1t.tile([128, tt], F32, name="rec", tag="rec",
                                   bufs=2)
                    nc.vector.reciprocal(rec[:], rb[:])
                    nc.gpsimd.tensor_mul(qTn[st["h"]][:, ts], st["qr"][:],
                                         rec[:])
                else:  # k: per-block exp scales SM_SCALE/rms via PE transpose
                    rr = st["rr"]
                    rkp = p1ss.tile([128, 4], F32, name="rk_ps", tag="rk",
                                    bufs=1)
                    for b in range(4):
                        nc.tensor.matmul(rkp[:, b:b + 1],
                                         rr[0:1, 128 * b:128 * (b + 1)],
                                         smsc_f[:])
                    nc.vector.tensor_copy(rks[:, 4 * i:4 * i + 4], rkp[:])

            for u in range(len(units)):
                emit_proj_part0(u)
                if u >= 1:
                    emit_part1(u - 1)
                if u >= 2:
                    emit_part2(u - 2)
            nu = len(units)
            emit_part1(nu - 1)
            emit_part2(nu - 2)
            emit_part2(nu - 1)

        # ================= phase 2+3 shared SBUF =================
        with tc.tile_pool(name="p2m", bufs=1) as p2m:
            masks = []
            for r in range(4):
                m = p2m.tile([128, tt], BF16, name=f"mask{r}")
                nc.gpsimd.memset(m[:], 1.0)
                nc.gpsimd.affine_select(
                    out=m[:], in_=m[:], compare_op=ALU.is_ge, fill=0.0,
                    base=-128 * r, pattern=[[1, tt]], channel_multiplier=-1)
                masks.append(m)

            # wo tiles (DMA overlaps with attention)
            wo_sb = [p2m.tile([128, DH], BF16, name=f"wo{c}") for c in range(NKC)]
            for c in range(NKC):
                nc.gpsimd.dma_start(wo_sb[c][:], woT[128 * c:128 * (c + 1), :])

            # last tile's AG is split into two head-pair halves so the
            # final out-proj can start before the last heads finish
            ag_in = [dram.tile([DH, tt], BF16, name=f"ag_in{i}")
                     for i in range(nt - 1)]
            ag_out = [dram.tile([4 * DH, tt], BF16, name=f"ag_out{i}")
                      for i in range(nt - 1)]
            ag_in3 = [dram.tile([2 * 128, tt], BF16, name=f"ag_in3{p}")
                      for p in range(2)]
            ag_out3 = [dram.tile([4 * 2 * 128, tt], BF16, name=f"ag_out3{p}")
                       for p in range(2)]

            # ---------- phase 2: causal attention ----------
            with (
                tc.tile_pool(name="p2pt", bufs=6) as p2pt,
                tc.tile_pool(name="p2on", bufs=6) as p2on,
                tc.tile_pool(name="p2st", bufs=2, space="PSUM") as p2st,
                tc.tile_pool(name="p2o", bufs=1, space="PSUM") as p2o,
                tc.tile_pool(name="p2tp", bufs=1, space="PSUM") as p2tp,
                tc.tile_pool(name="p3y", bufs=1) as p3y,
                tc.tile_pool(name="p3t", bufs=6) as p3t,
            ):
                def emit_att_head(i, h):
                    nj = min(4 * i + 4, nkb)
                    # token-blocks t=0,1 and t=2,3 pair up in one PSUM
                    # bank each, as a single accumulation group per bank
                    o01 = p2o.tile([128, 258], F32, name="o01", tag="o01",
                                   bufs=2)
                    o23 = p2o.tile([128, 258], F32, name="o23", tag="o23",
                                   bufs=2)
                    o_of = {0: (o01, 0), 1: (o01, 129),
                            2: (o23, 0), 3: (o23, 129)}
                    for j in range(nj):
                        r = j - 4 * i
                        # diagonal blocks r=1,2,3: the first 128*r score
                        # columns are fully masked — skip them
                        off = 128 * r if r > 0 else 0
                        npr = tt - off
                        st = p2st.tile([128, tt], F32, name="st_ps")
                        nc.tensor.matmul(
                            st[:, 0:npr], kTn[:, 128 * j:128 * (j + 1)],
                            qTn[h][:, i * tt + off:(i + 1) * tt])
                        pt = p2pt.tile([128, tt], BF16, name="pt")
                        nc.scalar.activation(pt[:, 0:npr], st[:, 0:npr],
                                             AF.Exp,
                                             scale=rks[:, j:j + 1])
                        if r >= 0:
                            nc.vector.tensor_mul(pt[:, 0:npr], pt[:, 0:npr],
                                                 masks[r][:, off:tt])
                        for t in range(4):
                            if j <= 4 * i + t:
                                ot_ps, oc = o_of[t]
                                nc.tensor.matmul(
                                    ot_ps[:, oc:oc + 129],
                                    pt[:, 128 * t - off:128 * (t + 1) - off],
                                    v_aug[j][:],
                                    start=(j == 0 and t % 2 == 0),
                                    stop=(j == 4 * i + t and t % 2 == 1))
                    for t in range(4):
                        ot_ps, oc = o_of[t]
                        rec = p2on.tile([128, 1], F32, name="rec")
                        nc.vector.reciprocal(
                            rec[:], ot_ps[:, oc + 128:oc + 129])
                        o_n = p2on.tile([128, 128], BF16, name="o_n")
                        nc.vector.tensor_scalar_mul(
                            o_n[:], ot_ps[:, oc:oc + 128], rec[:])
                        tp = p2tp.tile([128, 128], BF16, name="o_tp")
                        nc.tensor.matmul(tp[:], o_n[:], ident_bf[:],
                                         is_transpose=True)
                        nc.vector.tensor_copy(
                            yT[h][:, i * tt + 128 * t:i * tt + 128 * (t + 1)],
                            tp[:])
                    if i == nt - 1:
                        # eager per-head staging; AG per head-pair
                        pair, ha = h // 2, h % 2
                        nc.sync.dma_start(
                            ag_in3[pair][128 * ha:128 * (ha + 1), :],
                            yT[h][:, i * tt:(i + 1) * tt])
                        if ha == 1 and not CFG["skip_ag"]:
                            nc.gpsimd.collective_compute(
                                "AllGather", ALU.bypass,
                                replica_groups=groups,
                                ins=[ag_in3[pair][:]],
                                outs=[ag_out3[pair][:]])

                def emit_ag(i):
                    if i >= nt - 1:
                        return  # last tile handled per head-pair
                    for h in range(HLOC):
                        nc.sync.dma_start(ag_in[i][128 * h:128 * (h + 1), :],
                                          yT[h][:, i * tt:(i + 1) * tt])
                    if not CFG["skip_ag"]:
                        nc.gpsimd.collective_compute(
                            "AllGather", ALU.bypass, replica_groups=groups,
                            ins=[ag_in[i][:]], outs=[ag_out[i][:]])

                proj_state = {}

                def proj_load_yf(i):
                    if i < nt - 1:
                        order = list(range(NKC))
                        srcs = [ag_out[i][128 * c:128 * (c + 1), :]
                                for c in order]
                    else:
                        # wave A (c%4 in 0,1 — from ag_out3[0]) first, so
                        # the final accumulation can start before AG3b lands
                        order = ([c for c in range(NKC) if c % 4 < 2]
                                 + [c for c in range(NKC) if c % 4 >= 2])
                        srcs = []
                        for c in order:
                            pair, hl = (0, c % 4) if c % 4 < 2 else (1, c % 4 - 2)
                            base = 256 * (c // 4) + 128 * hl
                            srcs.append(ag_out3[pair][base:base + 128, :])
                    yf = {}
                    for c, src in zip(order, srcs):
                        yc = p3y.tile([128, tt], BF16, name=f"yf{c}", tag="yf",
                                      bufs=2 * NKC)
                        nc.sync.dma_start(yc[:], src)
                        yf[c] = yc
                    proj_state[i] = (order, yf)

                def emit_proj_sub(i, t):
                    if CFG["phases"] < 3:
                        return
                    if i not in proj_state:
                        proj_load_yf(i)
                    order, yf = proj_state[i]
                    ps = p2st.tile([128, DH], F32, name="out_ps", tag="cp",
                                   bufs=1)
                    for k, c in enumerate(order):
                        nc.tensor.matmul(
                            ps[:], yf[c][:, 128 * t:128 * (t + 1)],
                            wo_sb[c][:],
                            start=(k == 0), stop=(k == NKC - 1))
                    ot = p3t.tile([128, DH], F32, name="ot")
                    nc.vector.tensor_copy(ot[:], ps[:])
                    nc.sync.dma_start(
                        out[i * tt + 128 * t:i * tt + 128 * (t + 1), :],
                        ot[:])

                # interleaved schedule: out-proj of tile i rides inside the
                # attention of tile i+2 (its AllGather has long completed)
                if CFG["phases"] >= 2:
                    for i in range(nt):
                        for h in range(HLOC):
                            emit_att_head(i, h)
                            if i >= 2:
                                emit_proj_sub(i - 2, h)
                        emit_ag(i)
                    for t in range(4):
                        emit_proj_sub(nt - 2, t)
                    for t in range(4):
                        emit_proj_sub(nt - 1, t)


def rope_only(nc, tmp_pool, sw_pool, ss_pool, ps, cos, sin_s, out_ap,
              ones_col, eps_t, smsc, rks_blocks):
    """RoPE for k; writes roped (unnormalized) k to out_ap and the per-tk
    exp scales SM_SCALE/rms into rks_blocks ([128,1] each, via PE transpose
    of the [1, tt] reciprocal-rms row).

    RoPE is orthogonal per (t, d-pair), so rms(rope(x)) = rms(x): the rms
    path reads the pre-rope PSUM directly and runs concurrently with the
    rotation. The half-swap is two 64-partition DVE muls at partition
    offsets (no PE swap matmul, no staging copy).
    """
    ttl = ps.shape[-1]
    h2 = 64
    # rms path (from pre-rope ps)
    sq = tmp_pool.tile([128, ttl], F32R, name="sq", tag="sq")
    nc.scalar.activation(sq[:], ps[:], AF.Square)
    ss = ss_pool.tile([1, ttl], F32, name="ss_ps", tag="ss")
    nc.tensor.matmul(ss[:], ones_col[:], sq[:])
    sd = tmp_pool.tile([1, ttl], F32, name="sd", tag="sd")
    nc.scalar.activation(sd[:], ss[:], AF.Sqrt, scale=1.0 / 128.0,
                         bias=eps_t[:])
    rr = tmp_pool.tile([1, ttl], F32, name="rr", tag="rr")
    nc.vector.reciprocal(rr[:], sd[:])
    for b, rk in enumerate(rks_blocks):
        rkp = ss_pool.tile([128, 1], F32, name="rk_ps", tag="rb")
        nc.tensor.matmul(rkp[:], rr[0:1, 128 * b:128 * (b + 1)], smsc[:])
        nc.vector.tensor_copy(rk[:], rkp[:])
    # rotation path
    e1 = tmp_pool.tile([128, ttl], F32, name="e1", tag="e1")
    nc.vector.tensor_mul(e1[:], ps[:], cos)
    qr = tmp_pool.tile([128, ttl], F32, name="qr", tag="qr")
    nc.vector.tensor_mul(qr[0:h2, :], ps[h2:128, :], sin_s[0:h2, :])
    nc.vector.tensor_mul(qr[h2:128, :], ps[0:h2, :], sin_s[h2:128, :])
    nc.gpsimd.tensor_add(out_ap, e1[:], qr[:])


def rope_norm(nc, tmp_pool, sw_pool, ss_pool, ps, cos, sin_s, out_ap,
              ones_col, ones_row, eps_t):
    """RoPE + RMS-norm. ps: [128 d, tt] PSUM (pre-rope head), out_ap: SBUF.

    cos is [cos; cos] (rows duplicated), sin_s is [sin; -sin]. Rotation
    preserves the per-t norm, so the rms factor is computed from the
    pre-rope PSUM concurrently with the rotation, then applied once at
    the end: out = (ps*cos + swap(ps)*sin_s) * rsqrt(mean(ps^2)+eps).
    """
    ttl = ps.shape[-1]
    h2 = 64
    # rms path: sumsq over d via PE, rsqrt on ACT, bcast to 128 parts via PE
    sq = tmp_pool.tile([128, ttl], F32R, name="sq", tag="sq")
    nc.scalar.activation(sq[:], ps[:], AF.Square)
    ss = ss_pool.tile([1, ttl], F32, name="ss_ps", tag="ss")
    nc.tensor.matmul(ss[:], ones_col[:], sq[:])
    sd = tmp_pool.tile([1, ttl], F32R, name="sd", tag="sd")
    nc.scalar.activation(sd[:], ss[:], AF.Sqrt, scale=1.0 / 128.0,
                         bias=eps_t[:])
    rb = ss_pool.tile([128, ttl], F32, name="rb_ps", tag="rb")
    nc.tensor.matmul(rb[:], ones_row[:], sd[:])
    rec = tmp_pool.tile([128, ttl], F32, name="rec", tag="rec")
    nc.vector.reciprocal(rec[:], rb[:])
    # rotation path (reads PSUM directly; half-swap via partition offsets)
    e1 = tmp_pool.tile([128, ttl], F32, name="e1", tag="e1")
    nc.vector.tensor_mul(e1[:], ps[:], cos)
    qr = tmp_pool.tile([128, ttl], F32, name="qr", tag="qr")
    nc.vector.tensor_mul(qr[0:h2, :], ps[h2:128, :], sin_s[0:h2, :])
    nc.vector.tensor_mul(qr[h2:128, :], ps[0:h2, :], sin_s[h2:128, :])
    nc.gpsimd.tensor_add(qr[:], e1[:], qr[:])
    nc.vector.tensor_mul(out_ap, qr[:], rec[:])


_NC_CACHE = {}


def get_nc(t_seq=T, n_reps=1):
    key = (t_seq, n_reps)
    if key not in _NC_CACHE:
        _NC_CACHE[key] = build_nc(t_seq, n_reps)
    return _NC_CACHE[key]


def make_in_maps(x, cos, sin, Wq, Wk, Wv, Wo, t_seq=T):
    half = D // 2
    cosT = np.ascontiguousarray(cos.reshape(t_seq, half).T.astype(np.float32))
    sinT = np.ascontiguousarray(sin.reshape(t_seq, half).T.astype(np.float32))
    cos2 = np.concatenate([cosT, cosT], axis=0)
    sin2s = np.concatenate([sinT, -sinT], axis=0)
    wqTs, wkTs, wvTs, woTs = [], [], [], []
    for g in range(4):
        wqTs.append(np.ascontiguousarray(
            Wq[DH * g:DH * (g + 1), :].T.astype(BF16NP)))
        wkTs.append(np.ascontiguousarray(
            Wk[D * g:D * (g + 1), :].T.astype(BF16NP)))
        wvTs.append(np.ascontiguousarray(
            Wv[D * g:D * (g + 1), :].T.astype(BF16NP)))
        woTs.append(np.ascontiguousarray(
            Wo[DH * g:DH * (g + 1), :].T.astype(BF16NP)))
    xTs = [np.ascontiguousarray(x[b].T.astype(BF16NP)) for b in range(x.shape[0])]
    in_maps = []
    for c in range(N_CORES):
        b, g = c // 4, c % 4
        in_maps.append({
            "xT": xTs[b], "wqT": wqTs[g], "wkT": wkTs[g], "wvT": wvTs[g],
            "woT": woTs[g], "cos2": cos2, "sin2s": sin2s,
        })
    return in_maps


def kernel(x, cos, sin, Wq, Wk, Wv, Wo):
    x = np.asarray(x, dtype=np.float32)
    nc = get_nc(T)
    in_maps = make_in_maps(x, np.asarray(cos), np.asarray(sin),
                           np.asarray(Wq), np.asarray(Wk), np.asarray(Wv),
                           np.asarray(Wo), T)
    res = run_bass_kernel_spmd(nc, in_maps, core_ids=list(range(N_CORES)))
    outa = np.empty((B, T, C), dtype=np.float32)
    for c in range(N_CORES):
        b, g = c // 4, c % 4
        outa[b, :, DH * g:DH * (g + 1)] = res.results[c]["out"]
    return outa
